# revision 6
# baseline (speedup 1.0000x reference)
"""DeepFM forward kernel for 8 Trainium2 NeuronCores.

Strategy: data-parallel over the batch (2048 samples/core). The two embedding
tables are merged into one (F*V, 17) table (16 emb2 cols + 1 emb1 col) so one
indirect-DMA row fetch serves both the FM and deep parts. Per 128-sample tile,
39 indirect DMAs gather the (128, 663) row block; DVE applies the Xv scaling,
PE transposes to feature-major, small sel-matmuls produce the L2 norm over the
field dim and the FM second-order term, and a 3-layer MLP with training-mode
BatchNorm runs feature-major (contraction on partitions). Batch statistics are
combined across cores with tiny AllReduces. All partial sums (f1, f2, sum(h))
are accumulated into a single (1, n) PSUM row per sample group.

Perf notes vs the original baseline:
 - indices are pre-offset on the host (xi + f*V) so gathers start immediately,
 - weights are cast to bf16 on the host (no SWDGE cast DMAs on the Pool
   engine, which is saturated by the 624 indirect gathers),
 - the MLP loops are ordered m-outer / k-middle / g-inner so each weight
   block is loaded into the PE array once per 4 matmuls,
 - per-layer BN scale/shift vectors are packed into one (128, 24) input.
"""

import numpy as np
import ml_dtypes

import concourse.bass as bass
import concourse.mybir as mybir
import concourse.tile as tile
from concourse import bacc
from concourse.bass_utils import run_bass_kernel_spmd

N, F, V, E = 16384, 39, 100000, 16
W17 = E + 1            # merged table row width
FE = F * E             # 624
HID = 400
BN_EPS = 1e-5
NC = 8
NLOC = N // NC         # 2048
P = 128
NT = NLOC // P         # 16 tiles of 128 samples
S = 512                # sample group width for the MLP
NG = NLOC // S         # 4 groups
TPG = S // P           # tiles per group

KC = [128, 128, 128, 128, 112]          # K blocks of FE=624
KO = [0, 128, 256, 384, 512]
MB = [128, 128, 128, 16]                # blocks of HID=400
MO = [0, 128, 256, 384]

F32 = mybir.dt.float32
BF16 = mybir.dt.bfloat16
I32 = mybir.dt.int32
AF = mybir.ActivationFunctionType


def build_kernel(n_cores=NC, mlp_dt=BF16):
    nc = bacc.Bacc("TRN2", target_bir_lowering=False, debug=False,
                   num_devices=n_cores)

    dram = {}
    def din(name, shape, dt):
        dram[name] = nc.dram_tensor(name, shape, dt, kind="ExternalInput").ap()
        return dram[name]

    xi = din("xi", [NLOC, F], I32)          # pre-offset: xi + f*V
    xv = din("xv", [NLOC, F], F32)
    emb12 = din("emb12", [F * V, W17], F32)
    w1t = din("w1t", [FE, HID], BF16)
    w2t = din("w2t", [HID, HID], BF16)
    w3t = din("w3t", [HID, HID], BF16)
    gbt = din("gbt", [P, 24], F32)          # packed g/bt per layer/m-block
    bias = din("bias", [1, 1], F32)
    ident = din("ident", [P, P], F32)
    sel16 = din("sel16", [P, E], F32)
    sel16c = din("sel16c", [P, E], BF16)
    sel16t = din("sel16t", [E, P], F32)
    onesc = din("onesc", [P, 1], F32)
    onesb = din("onesb", [P, 1], BF16)
    half16 = din("half16", [E, 1], F32)
    out = nc.dram_tensor("out", [1, NLOC], F32, kind="ExternalOutput").ap()

    with tile.TileContext(nc) as tc:
        import contextlib
        with contextlib.ExitStack() as ctx:
            pers = ctx.enter_context(tc.tile_pool(name="pers", bufs=1))
            gout = ctx.enter_context(tc.tile_pool(name="gout", bufs=10))
            nrm = ctx.enter_context(tc.tile_pool(name="nrm", bufs=3))
            nrm6 = ctx.enter_context(tc.tile_pool(name="nrm6", bufs=5))
            actp = ctx.enter_context(tc.tile_pool(name="actp", bufs=1))
            osb = ctx.enter_context(tc.tile_pool(name="osb", bufs=2))
            stp = ctx.enter_context(tc.tile_pool(name="stp", bufs=1))
            dramp = ctx.enter_context(tc.tile_pool(name="dramp", bufs=1, space="DRAM"))
            zp = ctx.enter_context(tc.tile_pool(name="zp", bufs=2))

            ps_t = ctx.enter_context(tc.tile_pool(name="ps_t", bufs=1, space="PSUM"))
            ps_ss = ctx.enter_context(tc.tile_pool(name="ps_ss", bufs=1, space="PSUM"))
            ps_rep = ctx.enter_context(tc.tile_pool(name="ps_rep", bufs=1, space="PSUM"))
            ps_po = ctx.enter_context(tc.tile_pool(name="ps_po", bufs=1, space="PSUM"))
            ps_z = ctx.enter_context(tc.tile_pool(name="ps_z", bufs=4, space="PSUM"))

            # ---- constants / weights -------------------------------------
            ident_t = pers.tile([P, P], F32, tag="ident", name="ident")
            nc.sync.dma_start(out=ident_t[:], in_=ident[:])
            sel_t = pers.tile([P, E], F32, tag="sel16", name="sel16")
            nc.sync.dma_start(out=sel_t[:], in_=sel16[:])
            selc_t = pers.tile([P, E], mlp_dt, tag="sel16c", name="sel16c")
            nc.sync.dma_start(out=selc_t[:], in_=sel16c[:])
            selt_t = pers.tile([E, P], F32, tag="sel16t", name="sel16t")
            nc.sync.dma_start(out=selt_t[:], in_=sel16t[:])
            ones_t = pers.tile([P, 1], F32, tag="ones", name="ones")
            nc.sync.dma_start(out=ones_t[:], in_=onesc[:])
            onesb_t = pers.tile([P, 1], mlp_dt, tag="onesb", name="onesb")
            nc.sync.dma_start(out=onesb_t[:], in_=onesb[:])
            half_t = pers.tile([E, 1], F32, tag="half16", name="half16")
            nc.sync.dma_start(out=half_t[:], in_=half16[:])
            bias_t = pers.tile([1, 1], F32, tag="bias", name="bias")
            nc.sync.dma_start(out=bias_t[:], in_=bias[:])
            eps_t = pers.tile([P, 1], F32, tag="eps", name="eps")
            nc.vector.memset(eps_t[:], BN_EPS)
            gbt_t = pers.tile([P, 24], F32, tag="gbt", name="gbt")
            nc.sync.dma_start(out=gbt_t[:], in_=gbt[:])
            # views: layer l, block m
            g_sb = [[gbt_t[:MB[m], l * 8 + m * 2:l * 8 + m * 2 + 1] for m in range(4)]
                    for l in range(3)]
            bt_sb = [[gbt_t[:MB[m], l * 8 + m * 2 + 1:l * 8 + m * 2 + 2] for m in range(4)]
                     for l in range(3)]

            w1sb = []
            for c in range(5):
                t = pers.tile([KC[c], HID], mlp_dt, tag=f"w1c{c}", name=f"w1c{c}")
                nc.sync.dma_start(out=t[:], in_=w1t[KO[c]:KO[c] + KC[c], :])
                w1sb.append(t)
            w2sb, w3sb = [], []
            for k in range(4):
                t = pers.tile([MB[k], HID], mlp_dt, tag=f"w2c{k}", name=f"w2c{k}")
                nc.sync.dma_start(out=t[:], in_=w2t[MO[k]:MO[k] + MB[k], :])
                w2sb.append(t)
                t = pers.tile([MB[k], HID], mlp_dt, tag=f"w3c{k}", name=f"w3c{k}")
                nc.sync.dma_start(out=t[:], in_=w3t[MO[k]:MO[k] + MB[k], :])
                w3sb.append(t)

            # all per-tile indices + xv up front (one DMA each)
            xi_all = pers.tile([P, NT * F], I32, tag="xi_all", name="xi_all")
            nc.sync.dma_start(
                out=xi_all[:].rearrange("p (t f) -> p t f", f=F),
                in_=xi[:].rearrange("(t p) f -> p t f", p=P))
            xv_all = pers.tile([P, NT * F], F32, tag="xv_all", name="xv_all")
            nc.sync.dma_start(
                out=xv_all[:].rearrange("p (t f) -> p t f", f=F),
                in_=xv[:].rearrange("(t p) f -> p t f", p=P))

            # persistent activations
            hbuf = [pers.tile([KC[c], NLOC], mlp_dt, tag=f"hbuf{c}", name=f"hbuf{c}") for c in range(5)]
            fsum = pers.tile([1, NLOC], F32, tag="fsum", name="fsum")
            def ztile(m):
                return zp.tile([MB[m], NLOC], mlp_dt, tag=f"zb{m}", name=f"zb{m}")
            st = [[stp.tile([MB[m], NG, 6], F32, tag=f"st{l}m{m}", name=f"st{l}m{m}") for m in range(4)]
                  for l in range(3)]
            ab_scale = [[stp.tile([MB[m], 1], F32, tag=f"av{l}m{m}", name=f"av{l}m{m}") for m in range(4)]
                        for l in range(3)]
            ab_shift = [[stp.tile([MB[m], 1], F32, tag=f"cv{l}m{m}", name=f"cv{l}m{m}") for m in range(4)]
                        for l in range(3)]

            # collective bounce buffers
            arin = [dramp.tile([HID, 2], F32, tag=f"arin{l}", name=f"arin{l}") for l in range(3)]
            arout = [dramp.tile([HID, 2], F32, tag=f"arout{l}", name=f"arout{l}") for l in range(3)]

            # ---------------- phase A: gathers + FM/norm per tile ----------
            zcur = [None] * 4

            def norm_tile(t):
                tcols = slice(t * P, (t + 1) * P)
                xfull = gout.tile([P, F * W17], F32, tag="xfull", name="xfull")
                for f in range(F):
                    nc.gpsimd.indirect_dma_start(
                        out=xfull[:, f * W17:(f + 1) * W17],
                        out_offset=None,
                        in_=emb12[:],
                        in_offset=bass.IndirectOffsetOnAxis(
                            ap=xi_all[:, t * F + f:t * F + f + 1], axis=0),
                    )
                xv_t = xv_all[:, t * F:(t + 1) * F]
                x3 = xfull[:].rearrange("p (f w) -> p f w", w=W17)
                xvc = nrm.tile([P, FE], F32, tag="xvc", name="xvc")
                xvb = xv_t.unsqueeze(2).to_broadcast([P, F, E])
                nc.vector.tensor_tensor(
                    out=xvc[:].rearrange("p (f e) -> p f e", e=E),
                    in0=x3[:, :, 0:E], in1=xvb, op=mybir.AluOpType.mult)
                # first-order term
                f1v = nrm.tile([P, F], F32, tag="f1v", name="f1v")
                nc.vector.tensor_tensor(out=f1v[:], in0=x3[:, :, E],
                                        in1=xv_t, op=mybir.AluOpType.mult)
                f1s = nrm.tile([P, 1], F32, tag="f1s", name="f1s")
                nc.vector.reduce_sum(out=f1s[:], in_=f1v[:], axis=mybir.AxisListType.X)

                # transposes + squares
                xvt_sb, sq_sb = [], []
                for c in range(5):
                    kc = KC[c]
                    tp = ps_t.tile([P, P], F32, tag="tps", name="tps")
                    nc.tensor.transpose(out=tp[:kc, :], in_=xvc[:, KO[c]:KO[c] + kc],
                                        identity=ident_t[:])
                    xt = nrm6.tile([P, P], F32, tag="xvt", name="xvt")
                    nc.vector.tensor_copy(out=xt[:kc, :], in_=tp[:kc, :])
                    sq = nrm.tile([P, P], F32, tag="sq", name="sq")
                    nc.scalar.square(out=sq[:kc, :], in_=tp[:kc, :])
                    xvt_sb.append(xt)
                    sq_sb.append(sq)
                ss = ps_ss.tile([E, P], F32, tag="ss", name="ss")
                for c in range(5):
                    nc.tensor.matmul(ss[:], sel_t[:KC[c], :], sq_sb[c][:KC[c], :],
                                     start=(c == 0), stop=(c == 4))
                nrm_t = nrm.tile([E, P], F32, tag="nrmt", name="nrmt")
                nc.scalar.sqrt(out=nrm_t[:], in_=ss[:])
                nc.vector.tensor_scalar_max(out=nrm_t[:], in0=nrm_t[:], scalar1=1e-12)
                inv = nrm.tile([E, P], F32, tag="inv", name="inv")
                nc.vector.reciprocal(out=inv[:], in_=nrm_t[:])
                inv2 = nrm.tile([E, P], F32, tag="inv2", name="inv2")
                nc.vector.tensor_mul(out=inv2[:], in0=inv[:], in1=inv[:])
                ssn = nrm.tile([E, P], F32, tag="ssn", name="ssn")
                nc.vector.tensor_mul(out=ssn[:], in0=ss[:], in1=inv2[:])
                rep = ps_rep.tile([P, P], F32, tag="rep", name="rep")
                nc.tensor.matmul(rep[:], selt_t[:], inv[:], start=True, stop=True)
                rsb = nrm.tile([P, P], F32, tag="rsb", name="rsb")
                nc.vector.tensor_copy(out=rsb[:], in_=rep[:])
                for c in range(5):
                    kc = KC[c]
                    nc.vector.tensor_tensor(out=hbuf[c][:kc, tcols],
                                            in0=xvt_sb[c][:kc, :], in1=rsb[:kc, :],
                                            op=mybir.AluOpType.mult)
                sps = ps_ss.tile([E, P], F32, tag="ss", name="sps")
                for c in range(5):
                    nc.tensor.matmul(sps[:], selc_t[:KC[c], :], hbuf[c][:KC[c], tcols],
                                     start=(c == 0), stop=(c == 4))
                q = nrm.tile([E, P], F32, tag="q", name="q")
                nc.scalar.square(out=q[:], in_=sps[:])
                d = nrm.tile([E, P], F32, tag="d", name="d")
                nc.vector.tensor_sub(out=d[:], in0=q[:], in1=ssn[:])
                po = ps_po.tile([1, P], F32, tag="po", name="po")
                nc.tensor.matmul(po[:], f1s[:], ident_t[:], start=True, stop=False,
                                 skip_group_check=True)
                nc.tensor.matmul(po[:], half_t[:], d[:], start=False, stop=True,
                                 skip_group_check=True)
                nc.vector.tensor_copy(out=fsum[:, tcols], in_=po[:])

            def mlp_layer(l, src_get, wsb, kblocks):
                """full layer l: m-outer / k-middle / g-inner (weight reuse).
                src_get(kI, g) -> AP of the (KC, S) activation block."""
                nkb = len(kblocks)
                for m in range(4):
                    zps = [ps_z.tile([P, S], F32, tag="zps", name="zps")
                           for _ in range(NG)]
                    for kI in range(nkb):
                        for g in range(NG):
                            nc.tensor.matmul(zps[g][:MB[m], :],
                                             wsb[kI][:, MO[m]:MO[m] + MB[m]],
                                             src_get(kI, g),
                                             start=(kI == 0), stop=(kI == nkb - 1))
                    for g in range(NG):
                        gcols = slice(g * S, (g + 1) * S)
                        nc.vector.bn_stats(out=st[l][m][:MB[m], g, :],
                                           in_=zps[g][:MB[m], :])
                        nc.vector.tensor_copy(out=zcur[m][:MB[m], gcols],
                                              in_=zps[g][:MB[m], :])

            def finalize_stats(l):
                for m in range(4):
                    mm = MB[m]
                    mv = nrm.tile([P, 2], F32, tag="mv", name="mv")
                    nc.vector.bn_aggr(out=mv[:mm, :], in_=st[l][m][:mm])
                    pk = nrm.tile([P, 2], F32, tag="pk", name="pk")
                    nc.vector.tensor_copy(out=pk[:mm, 0:1], in_=mv[:mm, 0:1])
                    tmp = nrm.tile([P, 1], F32, tag="tmp", name="tmp")
                    nc.vector.tensor_mul(out=tmp[:mm], in0=mv[:mm, 0:1], in1=mv[:mm, 0:1])
                    nc.vector.tensor_add(out=pk[:mm, 1:2], in0=mv[:mm, 1:2], in1=tmp[:mm])
                    nc.sync.dma_start(out=arin[l][MO[m]:MO[m] + mm, :], in_=pk[:mm, :])
                nc.gpsimd.collective_compute(
                    "AllReduce", mybir.AluOpType.add,
                    replica_groups=[list(range(n_cores))],
                    ins=[arin[l][:]], outs=[arout[l][:]],
                )
                for m in range(4):
                    mm = MB[m]
                    sm = nrm.tile([P, 2], F32, tag="sm", name="sm")
                    nc.sync.dma_start(out=sm[:mm, :], in_=arout[l][MO[m]:MO[m] + mm, :])
                    mu = nrm.tile([P, 1], F32, tag="mu", name="mu")
                    nc.vector.tensor_scalar_mul(out=mu[:mm], in0=sm[:mm, 0:1], scalar1=1.0 / n_cores)
                    e2 = nrm.tile([P, 1], F32, tag="e2", name="e2")
                    nc.vector.tensor_scalar_mul(out=e2[:mm], in0=sm[:mm, 1:2], scalar1=1.0 / n_cores)
                    var = nrm.tile([P, 1], F32, tag="var", name="var")
                    nc.vector.tensor_mul(out=var[:mm], in0=mu[:mm], in1=mu[:mm])
                    nc.vector.tensor_sub(out=var[:mm], in0=e2[:mm], in1=var[:mm])
                    sd = nrm.tile([P, 1], F32, tag="sd", name="sd")
                    nc.scalar.activation(out=sd[:mm], in_=var[:mm], func=AF.Sqrt,
                                         bias=eps_t[:mm], scale=1.0)
                    ri = nrm.tile([P, 1], F32, tag="ri", name="ri")
                    nc.vector.reciprocal(out=ri[:mm], in_=sd[:mm])
                    nc.vector.tensor_mul(out=ab_scale[l][m][:mm], in0=g_sb[l][m],
                                         in1=ri[:mm])
                    tmp2 = nrm.tile([P, 1], F32, tag="tmp2", name="tmp2")
                    nc.vector.tensor_mul(out=tmp2[:mm], in0=mu[:mm],
                                         in1=ab_scale[l][m][:mm])
                    nc.vector.tensor_sub(out=ab_shift[l][m][:mm], in0=bt_sb[l][m],
                                         in1=tmp2[:mm])

            # ============ emit program ============
            for t in range(NT):
                norm_tile(t)
            z1 = [ztile(m) for m in range(4)]
            zcur = z1
            mlp_layer(0, lambda kI, g: hbuf[kI][:KC[kI], g * S:(g + 1) * S],
                      w1sb, KC)
            finalize_stats(0)

            z2 = [ztile(m) for m in range(4)]
            a1 = {}
            for g in range(NG):
                gcols = slice(g * S, (g + 1) * S)
                for k in range(4):
                    at = actp.tile([MB[k], S], mlp_dt, tag=f"a{k}g{g}", name=f"a{k}g{g}")
                    nc.scalar.activation(out=at[:], in_=z1[k][:MB[k], gcols],
                                         func=AF.Relu, bias=ab_shift[0][k][:MB[k]],
                                         scale=ab_scale[0][k][:MB[k]])
                    a1[(k, g)] = at
            zcur = z2
            mlp_layer(1, lambda kI, g: a1[(kI, g)][:], w2sb, MB)
            finalize_stats(1)

            z3 = [ztile(m) for m in range(4)]
            a2 = {}
            for g in range(NG):
                gcols = slice(g * S, (g + 1) * S)
                for k in range(4):
                    at = actp.tile([MB[k], S], mlp_dt, tag=f"b{k}g{g}", name=f"b{k}g{g}")
                    nc.scalar.activation(out=at[:], in_=z2[k][:MB[k], gcols],
                                         func=AF.Relu, bias=ab_shift[1][k][:MB[k]],
                                         scale=ab_scale[1][k][:MB[k]])
                    a2[(k, g)] = at
            zcur = z3
            mlp_layer(2, lambda kI, g: a2[(kI, g)][:], w3sb, MB)
            finalize_stats(2)

            for g in range(NG):
                gcols = slice(g * S, (g + 1) * S)
                po2 = ps_po.tile([1, S], F32, tag="po", name="po")
                for m in range(4):
                    at = actp.tile([MB[m], S], mlp_dt, tag=f"c{m}", name=f"c{m}")
                    nc.scalar.activation(out=at[:], in_=z3[m][:MB[m], gcols],
                                         func=AF.Relu, bias=ab_shift[2][m][:MB[m]],
                                         scale=ab_scale[2][m][:MB[m]])
                    nc.tensor.matmul(po2[:], onesb_t[:MB[m], :], at[:],
                                     start=(m == 0), stop=(m == 3))
                ot = osb.tile([1, S], F32, tag="ot", name="ot")
                nc.vector.tensor_add(out=ot[:], in0=po2[:], in1=fsum[:, gcols])
                nc.vector.tensor_scalar(out=ot[:], in0=ot[:], scalar1=bias_t[:, :],
                                        scalar2=None, op0=mybir.AluOpType.add)
                nc.sync.dma_start(out=out[:, gcols], in_=ot[:])

    nc.compile()
    return nc


def _prep_inputs(Xi, Xv, emb1, emb2, W1, W2, W3, g1, bt1, g2, bt2, g3, bt3, bias):
    emb12 = np.concatenate(
        [np.asarray(emb2, np.float32).reshape(F * V, E),
         np.asarray(emb1, np.float32).reshape(F * V, 1)], axis=1)
    emb12 = np.ascontiguousarray(emb12)
    gbt = np.zeros((P, 24), np.float32)
    for l, (g, bt) in enumerate(((g1, bt1), (g2, bt2), (g3, bt3))):
        g = np.asarray(g, np.float32); bt = np.asarray(bt, np.float32)
        for m in range(4):
            mm = MB[m]
            gbt[:mm, l * 8 + m * 2] = g[MO[m]:MO[m] + mm]
            gbt[:mm, l * 8 + m * 2 + 1] = bt[MO[m]:MO[m] + mm]
    shared = {
        "emb12": emb12,
        "w1t": np.ascontiguousarray(np.asarray(W1, np.float32).T).astype(ml_dtypes.bfloat16),
        "w2t": np.ascontiguousarray(np.asarray(W2, np.float32).T).astype(ml_dtypes.bfloat16),
        "w3t": np.ascontiguousarray(np.asarray(W3, np.float32).T).astype(ml_dtypes.bfloat16),
        "gbt": gbt,
        "bias": np.asarray(bias, np.float32).reshape(1, 1),
        "ident": np.eye(P, dtype=np.float32),
        "sel16": (np.arange(P)[:, None] % E == np.arange(E)[None, :]).astype(np.float32),
        "sel16c": (np.arange(P)[:, None] % E == np.arange(E)[None, :]).astype(ml_dtypes.bfloat16),
        "sel16t": (np.arange(P)[None, :] % E == np.arange(E)[:, None]).astype(np.float32),
        "onesc": np.ones((P, 1), np.float32),
        "onesb": np.ones((P, 1), ml_dtypes.bfloat16),
        "half16": np.full((E, 1), 0.5, np.float32),
    }
    xi32 = (np.asarray(Xi).reshape(N, F).astype(np.int64)
            + (np.arange(F, dtype=np.int64) * V)[None, :]).astype(np.int32)
    xvf = np.asarray(Xv, np.float32)
    in_maps = []
    for c in range(NC):
        rows = slice(c * NLOC, (c + 1) * NLOC)
        m = dict(shared)
        m["xi"] = np.ascontiguousarray(xi32[rows])
        m["xv"] = np.ascontiguousarray(xvf[rows])
        in_maps.append(m)
    return in_maps


_NC_CACHE = {}


def kernel(Xi, Xv, emb1, emb2, W1, b1, g1, bt1, W2, b2, g2, bt2, W3, b3, g3,
           bt3, bias, _trace=False, _tmpdir=None):
    # b1/b2/b3 cancel inside training-mode BatchNorm (z - mean(z) is
    # bias-invariant), so they are accepted but unused.
    if "nc" not in _NC_CACHE:
        _NC_CACHE["nc"] = build_kernel()
    nc = _NC_CACHE["nc"]
    in_maps = _prep_inputs(Xi, Xv, emb1, emb2, W1, W2, W3,
                           g1, bt1, g2, bt2, g3, bt3, bias)
    res = run_bass_kernel_spmd(nc, in_maps, core_ids=list(range(NC)),
                               trace=_trace, tmpdir=_tmpdir)
    outp = np.concatenate([res.results[c]["out"].reshape(NLOC) for c in range(NC)])
    kernel.last_exec_time_ns = res.exec_time_ns
    return outp


# revision 19
# speedup vs baseline: 1.8062x; 1.8062x over previous
"""DeepFM forward kernel for 8 Trainium2 NeuronCores.

Strategy: data-parallel over the batch (2048 samples/core). The two embedding
tables are merged into one (F*V, 17) table (16 emb2 cols + 1 emb1 col) so one
indirect-DMA row fetch serves both the FM and deep parts. Per 128-sample tile,
39 indirect DMAs gather the (128, 663) row block; DVE applies the Xv scaling,
PE transposes to feature-major, small sel-matmuls produce the L2 norm over the
field dim and the FM second-order term, and a 3-layer MLP with training-mode
BatchNorm runs feature-major (contraction on partitions). Batch statistics are
combined across cores with tiny AllReduces. All partial sums (f1, f2, sum(h))
are accumulated into a single (1, n) PSUM row per sample group.

Perf notes vs the original baseline:
 - indices are pre-offset on the host (xi + f*V) so gathers start immediately,
 - weights are cast to bf16 on the host (no SWDGE cast DMAs on the Pool
   engine, which is saturated by the 624 indirect gathers),
 - the MLP loops are ordered m-outer / k-middle / g-inner so each weight
   block is loaded into the PE array once per 4 matmuls,
 - per-layer BN scale/shift vectors are packed into one (128, 24) input.
"""

import numpy as np
import ml_dtypes

import concourse.bass as bass
import concourse.mybir as mybir
import concourse.tile as tile
from concourse import bacc
from concourse.bass_utils import run_bass_kernel_spmd

N, F, V, E = 16384, 39, 100000, 16
W17 = E + 1            # merged table row width
FE = F * E             # 624
HID = 400
BN_EPS = 1e-5
NC = 8
NLOC = N // NC         # 2048
P = 128
NT = NLOC // P         # 16 tiles of 128 samples
S = 512                # sample group width for the MLP
NG = NLOC // S         # 4 groups
TPG = S // P           # tiles per group

KC = [128, 128, 128, 128, 112]          # K blocks of FE=624
KO = [0, 128, 256, 384, 512]
MB = [128, 128, 128, 16]                # blocks of HID=400
MO = [0, 128, 256, 384]

F32 = mybir.dt.float32
BF16 = mybir.dt.bfloat16
I32 = mybir.dt.int32
AF = mybir.ActivationFunctionType


NH = 2                 # sample halves (pipeline the gather vs norm compute)
SH = NLOC // NH        # 1024 samples per half
SEG = SH               # gather slots per (field, half) = one half, sample order
SEGC = SEG // 16       # idx columns per segment (64)
EP64 = 64              # padded table row: 64 f32 = 256 B
NSEG = F * NH


def build_kernel(n_cores=NC, mlp_dt=BF16):
    nc = bacc.Bacc("TRN2", target_bir_lowering=False, debug=False,
                   num_devices=n_cores, num_swdge_queues=4)

    dram = {}
    def din(name, shape, dt):
        dram[name] = nc.dram_tensor(name, shape, dt, kind="ExternalInput").ap()
        return dram[name]

    xv = din("xv", [NLOC, F], F32)
    embp = din("embp", [F * V, EP64], F32)
    idxg = din("idxg", [P, NSEG * SEGC], mybir.dt.int16)
    w1t = din("w1t", [FE, HID], BF16)
    w2t = din("w2t", [HID, HID], BF16)
    w3t = din("w3t", [HID, HID], BF16)
    gbt = din("gbt", [P, 24], F32)          # packed g/bt per layer/m-block
    bias = din("bias", [1, 1], F32)
    ident = din("ident", [P, P], F32)
    sel16 = din("sel16", [P, E], F32)
    sel16c = din("sel16c", [P, E], BF16)
    sel16t = din("sel16t", [E, P], F32)
    onesc = din("onesc", [P, 1], F32)
    onesb = din("onesb", [P, 1], BF16)
    half16 = din("half16", [E, 1], F32)
    out = nc.dram_tensor("out", [1, NLOC], F32, kind="ExternalOutput").ap()

    with tile.TileContext(nc) as tc:
        import contextlib
        with contextlib.ExitStack() as ctx:
            pers = ctx.enter_context(tc.tile_pool(name="pers", bufs=1))
            gpool = ctx.enter_context(tc.tile_pool(name="gpool", bufs=8))
            cpool = ctx.enter_context(tc.tile_pool(name="cpool", bufs=8))
            nrm = ctx.enter_context(tc.tile_pool(name="nrm", bufs=3))
            nrm6 = ctx.enter_context(tc.tile_pool(name="nrm6", bufs=5))
            actp = ctx.enter_context(tc.tile_pool(name="actp", bufs=1))
            osb = ctx.enter_context(tc.tile_pool(name="osb", bufs=2))
            stp = ctx.enter_context(tc.tile_pool(name="stp", bufs=1))
            dramp = ctx.enter_context(tc.tile_pool(name="dramp", bufs=1, space="DRAM"))
            zp = ctx.enter_context(tc.tile_pool(name="zp", bufs=2))

            ps_t = ctx.enter_context(tc.tile_pool(name="ps_t", bufs=1, space="PSUM"))
            ps_ss = ctx.enter_context(tc.tile_pool(name="ps_ss", bufs=1, space="PSUM"))
            ps_rep = ctx.enter_context(tc.tile_pool(name="ps_rep", bufs=1, space="PSUM"))
            ps_po = ctx.enter_context(tc.tile_pool(name="ps_po", bufs=1, space="PSUM"))
            ps_z = ctx.enter_context(tc.tile_pool(name="ps_z", bufs=4, space="PSUM"))

            # ---- constants / weights -------------------------------------
            ident_t = pers.tile([P, P], F32, tag="ident", name="ident")
            nc.sync.dma_start(out=ident_t[:], in_=ident[:])
            sel_t = pers.tile([P, E], F32, tag="sel16", name="sel16")
            nc.sync.dma_start(out=sel_t[:], in_=sel16[:])
            selc_t = pers.tile([P, E], mlp_dt, tag="sel16c", name="sel16c")
            nc.sync.dma_start(out=selc_t[:], in_=sel16c[:])
            selt_t = pers.tile([E, P], F32, tag="sel16t", name="sel16t")
            nc.sync.dma_start(out=selt_t[:], in_=sel16t[:])
            ones_t = pers.tile([P, 1], F32, tag="ones", name="ones")
            nc.sync.dma_start(out=ones_t[:], in_=onesc[:])
            onesb_t = pers.tile([P, 1], mlp_dt, tag="onesb", name="onesb")
            nc.sync.dma_start(out=onesb_t[:], in_=onesb[:])
            half_t = pers.tile([E, 1], F32, tag="half16", name="half16")
            nc.sync.dma_start(out=half_t[:], in_=half16[:])
            bias_t = pers.tile([1, 1], F32, tag="bias", name="bias")
            nc.sync.dma_start(out=bias_t[:], in_=bias[:])
            eps_t = pers.tile([P, 1], F32, tag="eps", name="eps")
            nc.vector.memset(eps_t[:], BN_EPS)
            gbt_t = pers.tile([P, 24], F32, tag="gbt", name="gbt")
            nc.sync.dma_start(out=gbt_t[:], in_=gbt[:])
            # views: layer l, block m
            g_sb = [[gbt_t[:MB[m], l * 8 + m * 2:l * 8 + m * 2 + 1] for m in range(4)]
                    for l in range(3)]
            bt_sb = [[gbt_t[:MB[m], l * 8 + m * 2 + 1:l * 8 + m * 2 + 2] for m in range(4)]
                     for l in range(3)]

            w1sb = []
            for c in range(5):
                t = pers.tile([KC[c], HID], mlp_dt, tag=f"w1c{c}", name=f"w1c{c}")
                nc.sync.dma_start(out=t[:], in_=w1t[KO[c]:KO[c] + KC[c], :])
                w1sb.append(t)
            w2sb, w3sb = [], []
            for k in range(4):
                t = pers.tile([MB[k], HID], mlp_dt, tag=f"w2c{k}", name=f"w2c{k}")
                nc.sync.dma_start(out=t[:], in_=w2t[MO[k]:MO[k] + MB[k], :])
                w2sb.append(t)
                t = pers.tile([MB[k], HID], mlp_dt, tag=f"w3c{k}", name=f"w3c{k}")
                nc.sync.dma_start(out=t[:], in_=w3t[MO[k]:MO[k] + MB[k], :])
                w3sb.append(t)

            # xv up front (one DMA)
            xv_all = pers.tile([P, NT * F], F32, tag="xv_all", name="xv_all")
            nc.sync.dma_start(
                out=xv_all[:].rearrange("p (t f) -> p t f", f=F),
                in_=xv[:].rearrange("(t p) f -> p t f", p=P))
            # gather/scatter index lists (host-sorted per field by vocab window)
            idxg_t = pers.tile([P, NSEG * SEGC], mybir.dt.int16,
                               tag="idxg", name="idxg")
            nc.sync.dma_start(out=idxg_t[:], in_=idxg[:])
            # gathered embeddings, all tiles x fields: [p, t, f, 17]
            xbig = pers.tile([P, NT * F * W17], F32, tag="xbig", name="xbig")
            xbig4 = xbig[:].rearrange("p (t f w) -> p t f w", f=F, w=W17)

            # persistent activations
            hbuf = [pers.tile([KC[c], NLOC], mlp_dt, tag=f"hbuf{c}", name=f"hbuf{c}") for c in range(5)]
            fsum = pers.tile([1, NLOC], F32, tag="fsum", name="fsum")
            def ztile(m):
                return zp.tile([MB[m], NLOC], mlp_dt, tag=f"zb{m}", name=f"zb{m}")
            st = [[stp.tile([MB[m], NG, 6], F32, tag=f"st{l}m{m}", name=f"st{l}m{m}") for m in range(4)]
                  for l in range(3)]
            ab_scale = [[stp.tile([MB[m], 1], F32, tag=f"av{l}m{m}", name=f"av{l}m{m}") for m in range(4)]
                        for l in range(3)]
            ab_shift = [[stp.tile([MB[m], 1], F32, tag=f"cv{l}m{m}", name=f"cv{l}m{m}") for m in range(4)]
                        for l in range(3)]

            # collective bounce buffers
            arin = [dramp.tile([HID, 2], F32, tag=f"arin{l}", name=f"arin{l}") for l in range(3)]
            arout = [dramp.tile([HID, 2], F32, tag=f"arout{l}", name=f"arout{l}") for l in range(3)]

            # ---------------- phase A: gathers + FM/norm per tile ----------
            zcur = [None] * 4

            qrr = [0]

            def gather_field(f, h):
                # one gather per (field, half): slots are in sample order, so
                # slot j*128+p of half h = sample h*SH + j*128 + p = tile
                # (h*TPH + j), partition p.
                seg = f * NH + h
                tph = SH // P
                g = gpool.tile([P, (SEG // P) * EP64], F32, tag="g", name="g")
                g3 = g[:].rearrange("p (j e) -> p j e", e=EP64)
                nc.gpsimd.dma_gather(
                    out_ap=g3,
                    in_ap=embp[f * V:(f + 1) * V, :],
                    idxs_ap=idxg_t[:, seg * SEGC:(seg + 1) * SEGC],
                    num_idxs=SEG,
                    num_idxs_reg=SEG,
                    elem_size=EP64,
                    single_packet=False,
                    queue_num=qrr[0],
                )
                qrr[0] = (qrr[0] + 1) % 4
                nc.vector.tensor_copy(
                    out=xbig4[:, h * tph:(h + 1) * tph, f, :],
                    in_=g3[:, :, 0:W17])

            def norm_tile(t):
                tcols = slice(t * P, (t + 1) * P)
                xv_t = xv_all[:, t * F:(t + 1) * F]
                x3 = xbig4[:, t]
                xvc = nrm.tile([P, FE], F32, tag="xvc", name="xvc")
                xvb = xv_t.unsqueeze(2).to_broadcast([P, F, E])
                nc.vector.tensor_tensor(
                    out=xvc[:].rearrange("p (f e) -> p f e", e=E),
                    in0=x3[:, :, 0:E], in1=xvb, op=mybir.AluOpType.mult)
                # first-order term
                f1v = nrm.tile([P, F], F32, tag="f1v", name="f1v")
                nc.vector.tensor_tensor(out=f1v[:], in0=x3[:, :, E],
                                        in1=xv_t, op=mybir.AluOpType.mult)
                f1s = nrm.tile([P, 1], F32, tag="f1s", name="f1s")
                nc.vector.reduce_sum(out=f1s[:], in_=f1v[:], axis=mybir.AxisListType.X)

                # transposes + squares
                xvt_sb, sq_sb = [], []
                for c in range(5):
                    kc = KC[c]
                    tp = ps_t.tile([P, P], F32, tag="tps", name="tps")
                    nc.tensor.transpose(out=tp[:kc, :], in_=xvc[:, KO[c]:KO[c] + kc],
                                        identity=ident_t[:])
                    xt = nrm6.tile([P, P], F32, tag="xvt", name="xvt")
                    nc.vector.tensor_copy(out=xt[:kc, :], in_=tp[:kc, :])
                    sq = nrm.tile([P, P], F32, tag="sq", name="sq")
                    nc.scalar.square(out=sq[:kc, :], in_=tp[:kc, :])
                    xvt_sb.append(xt)
                    sq_sb.append(sq)
                ss = ps_ss.tile([E, P], F32, tag="ss", name="ss")
                for c in range(5):
                    nc.tensor.matmul(ss[:], sel_t[:KC[c], :], sq_sb[c][:KC[c], :],
                                     start=(c == 0), stop=(c == 4))
                nrm_t = nrm.tile([E, P], F32, tag="nrmt", name="nrmt")
                nc.scalar.sqrt(out=nrm_t[:], in_=ss[:])
                nc.vector.tensor_scalar_max(out=nrm_t[:], in0=nrm_t[:], scalar1=1e-12)
                inv = nrm.tile([E, P], F32, tag="inv", name="inv")
                nc.vector.reciprocal(out=inv[:], in_=nrm_t[:])
                inv2 = nrm.tile([E, P], F32, tag="inv2", name="inv2")
                nc.vector.tensor_mul(out=inv2[:], in0=inv[:], in1=inv[:])
                ssn = nrm.tile([E, P], F32, tag="ssn", name="ssn")
                nc.vector.tensor_mul(out=ssn[:], in0=ss[:], in1=inv2[:])
                rep = ps_rep.tile([P, P], F32, tag="rep", name="rep")
                nc.tensor.matmul(rep[:], selt_t[:], inv[:], start=True, stop=True)
                rsb = nrm.tile([P, P], F32, tag="rsb", name="rsb")
                nc.vector.tensor_copy(out=rsb[:], in_=rep[:])
                for c in range(5):
                    kc = KC[c]
                    nc.vector.tensor_tensor(out=hbuf[c][:kc, tcols],
                                            in0=xvt_sb[c][:kc, :], in1=rsb[:kc, :],
                                            op=mybir.AluOpType.mult)
                sps = ps_ss.tile([E, P], F32, tag="ss", name="sps")
                for c in range(5):
                    nc.tensor.matmul(sps[:], selc_t[:KC[c], :], hbuf[c][:KC[c], tcols],
                                     start=(c == 0), stop=(c == 4))
                q = nrm.tile([E, P], F32, tag="q", name="q")
                nc.scalar.square(out=q[:], in_=sps[:])
                d = nrm.tile([E, P], F32, tag="d", name="d")
                nc.vector.tensor_sub(out=d[:], in0=q[:], in1=ssn[:])
                po = ps_po.tile([1, P], F32, tag="po", name="po")
                nc.tensor.matmul(po[:], f1s[:], ident_t[:], start=True, stop=False,
                                 skip_group_check=True)
                nc.tensor.matmul(po[:], half_t[:], d[:], start=False, stop=True,
                                 skip_group_check=True)
                nc.vector.tensor_copy(out=fsum[:, tcols], in_=po[:])

            def mlp_layer(l, src_get, wsb, kblocks):
                """full layer l: m-outer / k-middle / g-inner (weight reuse).
                src_get(kI, g) -> AP of the (KC, S) activation block."""
                nkb = len(kblocks)
                for m in range(4):
                    zps = [ps_z.tile([P, S], F32, tag="zps", name="zps")
                           for _ in range(NG)]
                    for kI in range(nkb):
                        for g in range(NG):
                            nc.tensor.matmul(zps[g][:MB[m], :],
                                             wsb[kI][:, MO[m]:MO[m] + MB[m]],
                                             src_get(kI, g),
                                             start=(kI == 0), stop=(kI == nkb - 1))
                    for g in range(NG):
                        gcols = slice(g * S, (g + 1) * S)
                        nc.vector.bn_stats(out=st[l][m][:MB[m], g, :],
                                           in_=zps[g][:MB[m], :])
                        nc.vector.tensor_copy(out=zcur[m][:MB[m], gcols],
                                              in_=zps[g][:MB[m], :])

            def finalize_stats(l):
                for m in range(4):
                    mm = MB[m]
                    mv = nrm.tile([P, 2], F32, tag="mv", name="mv")
                    nc.vector.bn_aggr(out=mv[:mm, :], in_=st[l][m][:mm])
                    pk = nrm.tile([P, 2], F32, tag="pk", name="pk")
                    nc.vector.tensor_copy(out=pk[:mm, 0:1], in_=mv[:mm, 0:1])
                    tmp = nrm.tile([P, 1], F32, tag="tmp", name="tmp")
                    nc.vector.tensor_mul(out=tmp[:mm], in0=mv[:mm, 0:1], in1=mv[:mm, 0:1])
                    nc.vector.tensor_add(out=pk[:mm, 1:2], in0=mv[:mm, 1:2], in1=tmp[:mm])
                    nc.sync.dma_start(out=arin[l][MO[m]:MO[m] + mm, :], in_=pk[:mm, :])
                nc.gpsimd.collective_compute(
                    "AllReduce", mybir.AluOpType.add,
                    replica_groups=[list(range(n_cores))],
                    ins=[arin[l][:]], outs=[arout[l][:]],
                )
                for m in range(4):
                    mm = MB[m]
                    sm = nrm.tile([P, 2], F32, tag="sm", name="sm")
                    nc.sync.dma_start(out=sm[:mm, :], in_=arout[l][MO[m]:MO[m] + mm, :])
                    mu = nrm.tile([P, 1], F32, tag="mu", name="mu")
                    nc.vector.tensor_scalar_mul(out=mu[:mm], in0=sm[:mm, 0:1], scalar1=1.0 / n_cores)
                    e2 = nrm.tile([P, 1], F32, tag="e2", name="e2")
                    nc.vector.tensor_scalar_mul(out=e2[:mm], in0=sm[:mm, 1:2], scalar1=1.0 / n_cores)
                    var = nrm.tile([P, 1], F32, tag="var", name="var")
                    nc.vector.tensor_mul(out=var[:mm], in0=mu[:mm], in1=mu[:mm])
                    nc.vector.tensor_sub(out=var[:mm], in0=e2[:mm], in1=var[:mm])
                    sd = nrm.tile([P, 1], F32, tag="sd", name="sd")
                    nc.scalar.activation(out=sd[:mm], in_=var[:mm], func=AF.Sqrt,
                                         bias=eps_t[:mm], scale=1.0)
                    ri = nrm.tile([P, 1], F32, tag="ri", name="ri")
                    nc.vector.reciprocal(out=ri[:mm], in_=sd[:mm])
                    nc.vector.tensor_mul(out=ab_scale[l][m][:mm], in0=g_sb[l][m],
                                         in1=ri[:mm])
                    tmp2 = nrm.tile([P, 1], F32, tag="tmp2", name="tmp2")
                    nc.vector.tensor_mul(out=tmp2[:mm], in0=mu[:mm],
                                         in1=ab_scale[l][m][:mm])
                    nc.vector.tensor_sub(out=ab_shift[l][m][:mm], in0=bt_sb[l][m],
                                         in1=tmp2[:mm])

            # ============ emit program ============
            for h in range(NH):
                for f in range(F):
                    gather_field(f, h)
            for t in range(NT):
                norm_tile(t)
            z1 = [ztile(m) for m in range(4)]
            zcur = z1
            mlp_layer(0, lambda kI, g: hbuf[kI][:KC[kI], g * S:(g + 1) * S],
                      w1sb, KC)
            finalize_stats(0)

            z2 = [ztile(m) for m in range(4)]
            for g in range(NG):
                gcols = slice(g * S, (g + 1) * S)
                for k in range(4):
                    nc.scalar.activation(out=z1[k][:MB[k], gcols],
                                         in_=z1[k][:MB[k], gcols],
                                         func=AF.Relu, bias=ab_shift[0][k][:MB[k]],
                                         scale=ab_scale[0][k][:MB[k]])
            zcur = z2
            mlp_layer(1, lambda kI, g: z1[kI][:MB[kI], g * S:(g + 1) * S], w2sb, MB)
            finalize_stats(1)

            z3 = [ztile(m) for m in range(4)]
            for g in range(NG):
                gcols = slice(g * S, (g + 1) * S)
                for k in range(4):
                    nc.scalar.activation(out=z2[k][:MB[k], gcols],
                                         in_=z2[k][:MB[k], gcols],
                                         func=AF.Relu, bias=ab_shift[1][k][:MB[k]],
                                         scale=ab_scale[1][k][:MB[k]])
            zcur = z3
            mlp_layer(2, lambda kI, g: z2[kI][:MB[kI], g * S:(g + 1) * S], w3sb, MB)
            finalize_stats(2)

            for g in range(NG):
                gcols = slice(g * S, (g + 1) * S)
                po2 = ps_po.tile([1, S], F32, tag="po", name="po")
                for m in range(4):
                    at = actp.tile([MB[m], S], mlp_dt, tag=f"c{m}", name=f"c{m}")
                    nc.scalar.activation(out=at[:], in_=z3[m][:MB[m], gcols],
                                         func=AF.Relu, bias=ab_shift[2][m][:MB[m]],
                                         scale=ab_scale[2][m][:MB[m]])
                    nc.tensor.matmul(po2[:], onesb_t[:MB[m], :], at[:],
                                     start=(m == 0), stop=(m == 3))
                ot = osb.tile([1, S], F32, tag="ot", name="ot")
                nc.vector.tensor_add(out=ot[:], in0=po2[:], in1=fsum[:, gcols])
                nc.vector.tensor_scalar(out=ot[:], in0=ot[:], scalar1=bias_t[:, :],
                                        scalar2=None, op0=mybir.AluOpType.add)
                nc.sync.dma_start(out=out[:, gcols], in_=ot[:])

    nc.compile()
    return nc


def _wrap16(v):
    """int16 slot list [n*16] -> wrapped [128, n] layout (slot k at [k%16, k//16],
    replicated across the 8 groups of 16 partitions)."""
    n = v.shape[0] // 16
    pat = v.reshape(n, 16).T
    out = np.empty((128, n), np.int16)
    for k in range(8):
        out[16 * k:16 * (k + 1)] = pat
    return out


def _prep_inputs(Xi, Xv, emb1, emb2, W1, W2, W3, g1, bt1, g2, bt2, g3, bt3, bias):
    xi_all = np.asarray(Xi).reshape(N, F).astype(np.int32)
    # Per-field vocab permutation: rows used by this batch first (= int16
    # addressable, at most N < 32768 of them), the rest after. Shared by all
    # cores; the table itself stays full-size and is gathered on device.
    newpos = np.empty((F, V), np.int32)
    nuse = np.empty(F, np.int64)
    for f in range(F):
        used = np.unique(xi_all[:, f])
        nuse[f] = used.shape[0]
        rest = np.setdiff1d(np.arange(V, dtype=np.int32), used,
                            assume_unique=True)
        order = np.concatenate([used.astype(np.int32), rest])
        inv = np.empty(V, np.int32)
        inv[order] = np.arange(V, dtype=np.int32)
        newpos[f] = inv
    assert nuse.max() < 32768
    emb2f = np.asarray(emb2, np.float32).reshape(F, V, E)
    emb1f = np.asarray(emb1, np.float32).reshape(F, V)
    embp = np.zeros((F * V, EP64), np.float32)
    e3 = embp.reshape(F, V, EP64)
    rng_rows = np.arange(V)
    for f in range(F):
        e3[f, newpos[f], :E] = emb2f[f]
        e3[f, newpos[f], E] = emb1f[f]
    gbt = np.zeros((P, 24), np.float32)
    for l, (g, bt) in enumerate(((g1, bt1), (g2, bt2), (g3, bt3))):
        g = np.asarray(g, np.float32); bt = np.asarray(bt, np.float32)
        for m in range(4):
            mm = MB[m]
            gbt[:mm, l * 8 + m * 2] = g[MO[m]:MO[m] + mm]
            gbt[:mm, l * 8 + m * 2 + 1] = bt[MO[m]:MO[m] + mm]
    shared = {
        "embp": embp,
        "w1t": np.ascontiguousarray(np.asarray(W1, np.float32).T).astype(ml_dtypes.bfloat16),
        "w2t": np.ascontiguousarray(np.asarray(W2, np.float32).T).astype(ml_dtypes.bfloat16),
        "w3t": np.ascontiguousarray(np.asarray(W3, np.float32).T).astype(ml_dtypes.bfloat16),
        "gbt": gbt,
        "bias": np.asarray(bias, np.float32).reshape(1, 1),
        "ident": np.eye(P, dtype=np.float32),
        "sel16": (np.arange(P)[:, None] % E == np.arange(E)[None, :]).astype(np.float32),
        "sel16c": (np.arange(P)[:, None] % E == np.arange(E)[None, :]).astype(ml_dtypes.bfloat16),
        "sel16t": (np.arange(P)[None, :] % E == np.arange(E)[:, None]).astype(np.float32),
        "onesc": np.ones((P, 1), np.float32),
        "onesb": np.ones((P, 1), ml_dtypes.bfloat16),
        "half16": np.full((E, 1), 0.5, np.float32),
    }
    xvf = np.asarray(Xv, np.float32)
    in_maps = []
    for c in range(NC):
        rows = slice(c * NLOC, (c + 1) * NLOC)
        xic = xi_all[rows]                     # (NLOC, F)
        idxg = np.empty((NSEG * SEG,), np.int16)
        for f in range(F):
            ridx = newpos[f][xic[:, f]]        # remapped, < nuse[f] < 32768
            for h in range(NH):
                base = (f * NH + h) * SEG
                idxg[base:base + SEG] = ridx[h * SH:(h + 1) * SH].astype(np.int16)
        m = dict(shared)
        m["idxg"] = _wrap16(idxg)
        m["xv"] = np.ascontiguousarray(xvf[rows])
        in_maps.append(m)
    return in_maps


_NC_CACHE = {}


def kernel(Xi, Xv, emb1, emb2, W1, b1, g1, bt1, W2, b2, g2, bt2, W3, b3, g3,
           bt3, bias, _trace=False, _tmpdir=None):
    # b1/b2/b3 cancel inside training-mode BatchNorm (z - mean(z) is
    # bias-invariant), so they are accepted but unused.
    if "nc" not in _NC_CACHE:
        _NC_CACHE["nc"] = build_kernel()
    nc = _NC_CACHE["nc"]
    in_maps = _prep_inputs(Xi, Xv, emb1, emb2, W1, W2, W3,
                           g1, bt1, g2, bt2, g3, bt3, bias)
    res = run_bass_kernel_spmd(nc, in_maps, core_ids=list(range(NC)),
                               trace=_trace, tmpdir=_tmpdir)
    outp = np.concatenate([res.results[c]["out"].reshape(NLOC) for c in range(NC)])
    kernel.last_exec_time_ns = res.exec_time_ns
    return outp
